# revision 2
# baseline (speedup 1.0000x reference)
"""LocalAffineAlignment Trainium2 kernel.

Shards B=16 across 8 NeuronCores (2 images/core, pure data parallel).

Per-core plan (all shapes hardcoded for B,C,H,W = 16,3,512,640, PATCH=8):
  - Each (img, ch) plane [512, 640] lives in SBUF as [128, 4, 640]:
    partition p, free (c, w) holds image row 128*c + p.
  - Masked per-patch moment sums (v, t*v, s*v, s^2*v, t*s*v) are computed as
    elementwise products (DVE/ACT) followed by 8-row group sums on the PE
    (fp32 matmuls with 0/1 "block shift" matrices, accumulated into one
    [128, 320] PSUM tile whose partition = h*64 + c*16 + patch_row, free =
    patch_col*8 + in-patch col for width-half h), then an 8->1 free-axis
    tensor_reduce on DVE.
  - Patch stats math runs on small [128, 3, 40] tiles (channels batched in
    the free dim; per-image scalars broadcast with 0-stride APs).
  - alpha/beta maps are upsampled back to planes with selection-matrix
    matmuls on the PE (PSUM) and evacuated to SBUF by the scalar engine.
  - aligned = alpha_full * warped + beta_full as full-plane DVE ops.
"""

import sys

if "/opt/trn_rl_repo" not in sys.path:
    sys.path.insert(0, "/opt/trn_rl_repo")

import numpy as np

B, C, H, W = 16, 3, 512, 640
NCORES = 8
PB = B // NCORES  # images per core
P = 128
NCH = H // P  # 4 row-chunks per plane
PATCH = 8
NH, NW = H // PATCH, W // PATCH  # 64, 80
HW2 = W // 2  # 320, width half
NWH = NW // 2  # 40 patch cols per half
EPS = 1e-6
MIN_VALID = float(int(0.1 * PATCH * PATCH))  # 6.0

# --- engine assignment knobs (tuned during iteration) ---
PH3_GPSIMD_CH = ()  # channels whose phase-3 mul/add run on gpsimd
TM_GPSIMD = False   # t*v product on gpsimd instead of DVE

_CACHE = {}


def _consts_np():
    """[128, 16*128] f32: 8 'bshift' then 8 'sel' matrices.

    bshift[(c,h)][k, m] = 1 iff m == h*64 + c*16 + k//8   (row-group sums)
    sel[(c,h)]          = bshift[(c,h)].T                  (8x row upsample)
    """
    mats = np.zeros((16, P, P), np.float32)
    for c in range(NCH):
        for h in range(2):
            i = c * 2 + h
            for k in range(P):
                mats[i, k, h * 64 + c * 16 + k // PATCH] = 1.0
    mats[8:] = np.transpose(mats[:8], (0, 2, 1))
    out = np.zeros((P, 16 * P), np.float32)
    for i in range(16):
        out[:, i * P : (i + 1) * P] = mats[i]
    return out


def _build():
    import concourse.bass as bass
    from concourse import bacc
    import concourse.tile as tile
    import concourse.mybir as mybir
    from contextlib import ExitStack

    f32 = mybir.dt.float32
    Alu = mybir.AluOpType
    AX = mybir.AxisListType.X

    nc = bacc.Bacc(
        "TRN2",
        target_bir_lowering=False,
        debug=False,
        enable_asserts=False,
        num_devices=NCORES,
    )

    t_in = nc.dram_tensor("target", [PB, C, H, W], f32, kind="ExternalInput")
    s_in = nc.dram_tensor("warped", [PB, C, H, W], f32, kind="ExternalInput")
    v_in = nc.dram_tensor("vmask", [PB, 1, H, W], f32, kind="ExternalInput")
    al_out = nc.dram_tensor("aligned", [PB, C, H, W], f32, kind="ExternalOutput")
    af_out = nc.dram_tensor("alphaf", [PB, 1, H, W], f32, kind="ExternalOutput")
    bf_out = nc.dram_tensor("betaf", [PB, 1, H, W], f32, kind="ExternalOutput")

    consts_dram = nc.inline_tensor(_consts_np(), name="consts")

    def plane_in(dram, b, ch):
        return dram.ap()[b, ch].rearrange("(c p) w -> p c w", p=P)

    def b3(x):
        # [128, 40] -> [128, 3, 40] via 0-stride middle dim
        return bass.AP(tensor=x.tensor, offset=x.offset, ap=[x.ap[0], [0, C], x.ap[1]])

    def b8(x):
        # [128, 40] -> [128, 40, 8] via 0-stride inner dim
        return bass.AP(tensor=x.tensor, offset=x.offset, ap=[x.ap[0], x.ap[1], [0, PATCH]])

    with ExitStack() as ctx:
        tc = ctx.enter_context(tile.TileContext(nc))
        singles = ctx.enter_context(tc.tile_pool(name="singles", bufs=1))
        s_pool = ctx.enter_context(tc.tile_pool(name="s_pool", bufs=2 * C))
        v_pool = ctx.enter_context(tc.tile_pool(name="v_pool", bufs=2))
        t_pool = ctx.enter_context(tc.tile_pool(name="t_pool", bufs=2))
        sm_pool = ctx.enter_context(tc.tile_pool(name="sm_pool", bufs=3))
        pr_pool = ctx.enter_context(tc.tile_pool(name="pr_pool", bufs=6))
        ab_pool = ctx.enter_context(tc.tile_pool(name="ab_pool", bufs=2))
        out_pool = ctx.enter_context(tc.tile_pool(name="out_pool", bufs=2))
        st_pool = ctx.enter_context(tc.tile_pool(name="st_pool", bufs=2))
        wd_pool = ctx.enter_context(tc.tile_pool(name="wd_pool", bufs=2))
        qp_pool = ctx.enter_context(tc.tile_pool(name="qp", bufs=1, space="PSUM"))
        vp_pool = ctx.enter_context(tc.tile_pool(name="vp", bufs=1, space="PSUM"))
        up_pool = ctx.enter_context(tc.tile_pool(name="up", bufs=1, space="PSUM"))

        consts = singles.tile([P, 16 * P], f32)
        nc.sync.dma_start(out=consts, in_=consts_dram.ap())

        def bw(i):  # bshift weight (c*2+h = i)
            return consts[:, i * P : (i + 1) * P]

        def sw(i):  # sel weight
            return consts[:, (8 + i) * P : (9 + i) * P]

        for b in range(PB):
            # ---- load mask, compute Sv (valid count per patch) ----
            v_t = v_pool.tile([P, NCH, W], f32, tag="v")
            nc.sync.dma_start(out=v_t, in_=plane_in(v_in, b, 0))

            ps_v = vp_pool.tile([P, HW2], f32, tag="psv")
            for c in range(NCH):
                for h in range(2):
                    nc.tensor.matmul(
                        ps_v,
                        bw(c * 2 + h),
                        v_t[:, c, h * HW2 : (h + 1) * HW2],
                        start=(c == 0 and h == 0),
                        stop=(c == NCH - 1 and h == 1),
                    )
            Sv = st_pool.tile([P, NWH], f32, tag="Sv")
            nc.vector.reduce_sum(out=Sv, in_=ps_v.rearrange("p (a b) -> p a b", b=PATCH), axis=AX)

            # ---- per-channel masked moment sums ----
            St = st_pool.tile([P, C, NWH], f32, tag="St")
            Ss = st_pool.tile([P, C, NWH], f32, tag="Ss")
            Sss = st_pool.tile([P, C, NWH], f32, tag="Sss")
            Sts = st_pool.tile([P, C, NWH], f32, tag="Sts")

            s_tiles = {}
            for ch in range(C):
                t_t = t_pool.tile([P, NCH, W], f32, tag="t")
                nc.sync.dma_start(out=t_t, in_=plane_in(t_in, b, ch))
                s_t = s_pool.tile([P, NCH, W], f32, tag="s")
                nc.sync.dma_start(out=s_t, in_=plane_in(s_in, b, ch))
                s_tiles[ch] = s_t

                ps_tm = qp_pool.tile([P, HW2], f32, tag="q0")
                ps_sm = qp_pool.tile([P, HW2], f32, tag="q1")
                ps_sq = qp_pool.tile([P, HW2], f32, tag="q2")
                ps_cr = qp_pool.tile([P, HW2], f32, tag="q3")

                for c in range(NCH):
                    smc = sm_pool.tile([P, W], f32, tag="sm")
                    nc.vector.tensor_mul(smc, s_t[:, c], v_t[:, c])
                    tmc = pr_pool.tile([P, W], f32, tag="pr")
                    if TM_GPSIMD:
                        nc.gpsimd.tensor_mul(tmc, t_t[:, c], v_t[:, c])
                    else:
                        nc.vector.tensor_mul(tmc, t_t[:, c], v_t[:, c])
                    crc = pr_pool.tile([P, W], f32, tag="pr")
                    nc.vector.tensor_mul(crc, t_t[:, c], smc)
                    sqc = pr_pool.tile([P, W], f32, tag="pr")
                    nc.scalar.square(sqc, smc)

                    st_ = c == 0
                    sp_ = c == NCH - 1
                    for h in range(2):
                        w0 = bw(c * 2 + h)
                        sl = slice(h * HW2, (h + 1) * HW2)
                        nc.tensor.matmul(ps_tm, w0, tmc[:, sl], start=st_ and h == 0, stop=sp_ and h == 1)
                        nc.tensor.matmul(ps_sm, w0, smc[:, sl], start=st_ and h == 0, stop=sp_ and h == 1)
                        nc.tensor.matmul(ps_sq, w0, sqc[:, sl], start=st_ and h == 0, stop=sp_ and h == 1)
                        nc.tensor.matmul(ps_cr, w0, crc[:, sl], start=st_ and h == 0, stop=sp_ and h == 1)

                nc.vector.reduce_sum(out=St[:, ch], in_=ps_tm.rearrange("p (a b) -> p a b", b=PATCH), axis=AX)
                nc.vector.reduce_sum(out=Ss[:, ch], in_=ps_sm.rearrange("p (a b) -> p a b", b=PATCH), axis=AX)
                nc.vector.reduce_sum(out=Sss[:, ch], in_=ps_sq.rearrange("p (a b) -> p a b", b=PATCH), axis=AX)
                nc.vector.reduce_sum(out=Sts[:, ch], in_=ps_cr.rearrange("p (a b) -> p a b", b=PATCH), axis=AX)

            # ---- patch stats -> alpha_map, beta_map  [128, 40] ----
            den = st_pool.tile([P, NWH], f32, tag="den")
            nc.vector.tensor_scalar(den, Sv, EPS, None, op0=Alu.add)
            r = st_pool.tile([P, NWH], f32, tag="r")
            nc.vector.reciprocal(r, den)
            pv = st_pool.tile([P, NWH], f32, tag="pv")
            nc.vector.tensor_scalar(pv, Sv, MIN_VALID, None, op0=Alu.is_ge)
            Ev = st_pool.tile([P, NWH], f32, tag="Ev")
            nc.vector.tensor_mul(Ev, Sv, r)
            g = st_pool.tile([P, NWH], f32, tag="g")
            nc.vector.tensor_scalar(g, Ev, -1.0, 2.0, op0=Alu.mult, op1=Alu.add)

            Et = st_pool.tile([P, C, NWH], f32, tag="Et")
            nc.vector.tensor_mul(Et, St, b3(r))
            Es = st_pool.tile([P, C, NWH], f32, tag="Es")
            nc.vector.tensor_mul(Es, Ss, b3(r))
            Ess = st_pool.tile([P, C, NWH], f32, tag="Ess")
            nc.vector.tensor_mul(Ess, Sss, b3(r))
            Ets = st_pool.tile([P, C, NWH], f32, tag="Ets")
            nc.vector.tensor_mul(Ets, Sts, b3(r))

            esg = st_pool.tile([P, C, NWH], f32, tag="esg")
            nc.vector.tensor_mul(esg, Es, b3(g))
            tmp = st_pool.tile([P, C, NWH], f32, tag="tmp")
            nc.vector.tensor_mul(tmp, Es, esg)
            svar = st_pool.tile([P, C, NWH], f32, tag="svar")
            nc.vector.tensor_sub(svar, Ess, tmp)
            tmp2 = st_pool.tile([P, C, NWH], f32, tag="tmp2")
            nc.vector.tensor_mul(tmp2, Et, esg)
            cov = st_pool.tile([P, C, NWH], f32, tag="cov")
            nc.vector.tensor_sub(cov, Ets, tmp2)

            sve = st_pool.tile([P, C, NWH], f32, tag="sve")
            nc.vector.tensor_scalar(sve, svar, EPS, None, op0=Alu.add)
            rv = st_pool.tile([P, C, NWH], f32, tag="rv")
            nc.vector.reciprocal(rv, sve)
            alpha = st_pool.tile([P, C, NWH], f32, tag="alpha")
            nc.vector.tensor_mul(alpha, cov, rv)
            ab_ = st_pool.tile([P, C, NWH], f32, tag="ab_")
            nc.vector.tensor_mul(ab_, alpha, Es)
            beta = st_pool.tile([P, C, NWH], f32, tag="beta")
            nc.vector.tensor_sub(beta, Et, ab_)

            asum = st_pool.tile([P, NWH], f32, tag="asum")
            nc.vector.tensor_add(asum, alpha[:, 0], alpha[:, 1])
            asum2 = st_pool.tile([P, NWH], f32, tag="asum2")
            nc.vector.tensor_add(asum2, asum, alpha[:, 2])
            am1 = st_pool.tile([P, NWH], f32, tag="am1")
            nc.vector.tensor_scalar(am1, asum2, 1.0 / 3.0, -1.0, op0=Alu.mult, op1=Alu.add)
            amp = st_pool.tile([P, NWH], f32, tag="amp")
            nc.vector.tensor_mul(amp, am1, pv)
            amap = st_pool.tile([P, NWH], f32, tag="amap")
            nc.vector.tensor_scalar(amap, amp, 1.0, None, op0=Alu.add)

            bsum = st_pool.tile([P, NWH], f32, tag="bsum")
            nc.vector.tensor_add(bsum, beta[:, 0], beta[:, 1])
            bsum2 = st_pool.tile([P, NWH], f32, tag="bsum2")
            nc.vector.tensor_add(bsum2, bsum, beta[:, 2])
            bmp = st_pool.tile([P, NWH], f32, tag="bmp")
            nc.vector.tensor_mul(bmp, bsum2, pv)
            bmap = st_pool.tile([P, NWH], f32, tag="bmap")
            nc.vector.tensor_scalar(bmap, bmp, 1.0 / 3.0, None, op0=Alu.mult)

            # ---- upsample maps to full planes via sel matmuls ----
            awide = wd_pool.tile([P, NWH, PATCH], f32, tag="aw")
            nc.vector.tensor_copy(out=awide, in_=b8(amap))
            bwide = wd_pool.tile([P, NWH, PATCH], f32, tag="bw")
            nc.vector.tensor_copy(out=bwide, in_=b8(bmap))
            awf = awide.rearrange("p a b -> p (a b)")
            bwf = bwide.rearrange("p a b -> p (a b)")

            af_t = ab_pool.tile([P, NCH, W], f32, tag="ab")
            bf_t = ab_pool.tile([P, NCH, W], f32, tag="ab")
            for c in range(NCH):
                for h in range(2):
                    i = c * 2 + h
                    sl = slice(h * HW2, (h + 1) * HW2)
                    pa = up_pool.tile([P, HW2], f32, tag="ua")
                    nc.tensor.matmul(pa, sw(i), awf, start=True, stop=True)
                    nc.scalar.copy(out=af_t[:, c, sl], in_=pa)
                    pb = up_pool.tile([P, HW2], f32, tag="ub")
                    nc.tensor.matmul(pb, sw(i), bwf, start=True, stop=True)
                    nc.scalar.copy(out=bf_t[:, c, sl], in_=pb)

            nc.sync.dma_start(out=plane_in(af_out, b, 0), in_=af_t)
            nc.sync.dma_start(out=plane_in(bf_out, b, 0), in_=bf_t)

            # ---- aligned = alpha_full * warped + beta_full ----
            for ch in range(C):
                out_t = out_pool.tile([P, NCH, W], f32, tag="out")
                if ch in PH3_GPSIMD_CH:
                    nc.gpsimd.tensor_mul(out_t, af_t, s_tiles[ch])
                    nc.gpsimd.tensor_add(out_t, out_t, bf_t)
                else:
                    nc.vector.tensor_mul(out_t, af_t, s_tiles[ch])
                    nc.vector.tensor_add(out_t, out_t, bf_t)
                nc.sync.dma_start(out=plane_in(al_out, b, ch), in_=out_t)

    nc.finalize()
    return nc


def _get_nc():
    if "nc" not in _CACHE:
        _CACHE["nc"] = _build()
    return _CACHE["nc"]


def _run(inputs, trace=False):
    from concourse.bass_utils import run_bass_kernel_spmd

    t = np.ascontiguousarray(inputs["target_img"], np.float32)
    s = np.ascontiguousarray(inputs["warped_source_img"], np.float32)
    v = np.ascontiguousarray(inputs["valid_mask"], np.float32)
    in_maps = []
    for i in range(NCORES):
        sl = slice(i * PB, (i + 1) * PB)
        in_maps.append(
            {
                "target": np.ascontiguousarray(t[sl]),
                "warped": np.ascontiguousarray(s[sl]),
                "vmask": np.ascontiguousarray(v[sl]),
            }
        )
    res = run_bass_kernel_spmd(
        _get_nc(), in_maps, core_ids=list(range(NCORES)), trace=trace
    )
    aligned = np.concatenate([r["aligned"] for r in res.results], axis=0)
    alphaf = np.concatenate([r["alphaf"] for r in res.results], axis=0)
    betaf = np.concatenate([r["betaf"] for r in res.results], axis=0)
    return (aligned, alphaf, betaf), res


def kernel(**inputs):
    out, _ = _run(inputs, trace=False)
    return out


def kernel_traced(**inputs):
    out, res = _run(inputs, trace=True)
    return out, res
